# revision 14
# baseline (speedup 1.0000x reference)
"""Trainium2 Bass kernel for nn_Biaffine (B=4, S=512, D=512, R=64).

Math: the reference computes
    left = einsum('bxi,irj,byj->bxyr', hf, U1, hb)
    out  = mean_y(left + rf[:, :, None] + rb[:, None] + bias)
The mean over y commutes with everything:
    mean_y(left)[b,x,r] = sum_ij hf[b,x,i] U1[i,r,j] hbbar[b,j],
    hbbar = mean_y(hb).
So out[b,x,r] = sum_i hf[b,x,i] * (V[b,i,r] + U2a[i,r]) + rbbar[b,r] + bias[r]
with V[b,i,r] = sum_j U1[i,r,j] hbbar[b,j], rbbar = hbbar @ U2b.

Sharding: tensor-parallel over r (dep_vec_dim): core c owns r in [8c, 8c+8).
Each core reads its U1 shard (8.4MB, the dominant traffic), full hf/hb (4MB
each), computes out[:, :, 8c:8c+8], host concatenates.
"""

import os
import sys

import numpy as np

try:
    import concourse.bass as bass  # noqa: F401
except ImportError:  # pragma: no cover
    sys.path.insert(0, "/opt/trn_rl_repo")

B, S, D, R = 4, 512, 512, 64
NCORES = 8
RB = R // NCORES  # 8 r's per core
P = 128
JC = D // P  # 4 j-chunks
IC = D // P  # 4 i-chunks
SY = S // NCORES  # 64 y's per core (mean partial, AllReduce'd)

# module-level knobs / results (test.py uses these; harness doesn't need them)
TRACE = os.environ.get("BASS_KERNEL_TRACE", "0") == "1"
LAST_RESULTS = None

_NC_CACHE = {}


def _build_nc(n_repeat=1):
    import concourse.bacc as bacc
    import concourse.mybir as mybir
    import concourse.tile as tile
    from concourse.masks import make_identity

    fp32 = mybir.dt.float32

    nc = bacc.Bacc("TRN2", target_bir_lowering=False, debug=False, num_devices=NCORES)

    hft_d = nc.dram_tensor("hft", [B, D, S], fp32, kind="ExternalInput")
    hb_d = nc.dram_tensor("hb", [B, SY, D], fp32, kind="ExternalInput")
    u1t_d = nc.dram_tensor("u1t", [D, RB, D], fp32, kind="ExternalInput")
    u2t_d = nc.dram_tensor("u2t", [2 * RB, D], fp32, kind="ExternalInput")
    bias_d = nc.dram_tensor("biasr", [1, RB], fp32, kind="ExternalInput")
    out_d = nc.dram_tensor("out", [B, RB, S], fp32, kind="ExternalOutput")

    with tile.TileContext(nc) as tc:
        with (
            tc.tile_pool(name="const", bufs=1) as cpool,
            tc.tile_pool(name="data", bufs=1) as dpool,
            tc.tile_pool(name="psum", bufs=6, space="PSUM") as ppool,
            tc.tile_pool(name="dram", bufs=1, space="DRAM") as drpool,
        ):
            identity = cpool.tile([P, P], fp32, tag="identity")
            make_identity(nc, identity)
            ones_y = cpool.tile([P, 1], fp32, tag="ones_y")
            nc.any.memset(ones_y, 1.0 / S)
            ones1 = cpool.tile([1, S], fp32, tag="ones1")
            nc.any.memset(ones1, 1.0)

            for _rep in range(n_repeat):
                _emit_body(
                    nc, dpool, ppool, drpool, fp32, identity, ones_y, ones1,
                    hft_d, hb_d, u1t_d, u2t_d, bias_d, out_d,
                )

    nc.compile()
    return nc


def _emit_body(
    nc, dpool, ppool, drpool, fp32, identity, ones_y, ones1,
    hft_d, hb_d, u1t_d, u2t_d, bias_d, out_d,
):
    import concourse.mybir as mybir
    if True:
        if True:
            u2raw = dpool.tile([2 * RB, D], fp32, tag="u2raw", bufs=2)
            u2sb = dpool.tile([P, IC, 2 * RB], fp32, tag="u2sb", bufs=2)
            bias_sb = dpool.tile([1, RB], fp32, tag="bias_sb", bufs=2)
            hbbarT = dpool.tile([P, JC * B], fp32, tag="hbbarT", bufs=2)
            rbb = dpool.tile([B, RB], fp32, tag="rbb", bufs=2)
            rbrow = dpool.tile([1, B * RB], fp32, tag="rbrow", bufs=2)
            vass = dpool.tile([P, IC, B, RB], fp32, tag="vass", bufs=2)

            # --- small inputs ---
            nc.sync.dma_start(out=u2raw, in_=u2t_d.ap())
            nc.sync.dma_start(out=bias_sb, in_=bias_d.ap())

            # --- hb y-slice load + partial mean over this core's 64 y's ---
            hbt = dpool.tile([SY, B, D], fp32, tag="hb", bufs=2)
            nc.sync.dma_start(out=hbt, in_=hb_d.ap().rearrange("b y j -> y b j"))

            # partial hbbarT[j, b] = (1/S) * sum_{y in slice} hb[b, y, j],
            # produced directly in [j, b] layout: lhsT = hb tile [y, j-slice]
            hbbarT_part = dpool.tile([P, JC * B], fp32, tag="hbbarT_part", bufs=2)
            for b in range(B):
                for jc in range(JC):
                    ps_hb = ppool.tile([P, 512], fp32, tag="ps")
                    nc.tensor.matmul(
                        ps_hb[:P, :1],
                        hbt[:, b, jc * P : (jc + 1) * P],
                        ones_y[:SY, :],
                        start=True,
                        stop=True,
                    )
                    nc.vector.tensor_copy(
                        out=hbbarT_part[:, jc * B + b : jc * B + b + 1],
                        in_=ps_hb[:P, :1],
                    )

            # --- AllReduce the 8KB partial means across the 8 cores ---
            ar_in = drpool.tile([P, JC * B], fp32, tag="ar_in")
            ar_out = drpool.tile([P, JC * B], fp32, tag="ar_out")
            nc.gpsimd.dma_start(ar_in[:], hbbarT_part)
            nc.gpsimd.collective_compute(
                "AllReduce",
                mybir.AluOpType.add,
                replica_groups=[list(range(NCORES))],
                ins=[ar_in.opt()],
                outs=[ar_out.opt()],
            )
            nc.sync.dma_start(out=hbbarT, in_=ar_out[:])

            # --- u2sb [d, (a|b)] via PE transpose: cols 0:RB=U2a[i,:], RB:2RB=U2b[j,:] ---
            for c in range(IC):
                ps_t = ppool.tile([P, 512], fp32, tag="ps")
                nc.tensor.transpose(
                    ps_t[:P, : 2 * RB],
                    u2raw[:, c * P : (c + 1) * P],
                    identity[: 2 * RB, : 2 * RB],
                )
                nc.vector.tensor_copy(out=u2sb[:, c, :], in_=ps_t[:P, : 2 * RB])

            # --- rbbar[b, r] = hbbar @ U2b (+ bias via K=1 ones-matmul) ---
            ps_rb = ppool.tile([P, 512], fp32, tag="ps")
            for jc in range(JC):
                nc.tensor.matmul(
                    ps_rb[:B, :RB],
                    hbbarT[:, jc * B : (jc + 1) * B],
                    u2sb[:, jc, RB : 2 * RB],
                    start=(jc == 0),
                    stop=False,
                )
            nc.tensor.matmul(
                ps_rb[:B, :RB], ones1[:1, :B], bias_sb, start=False, stop=True
            )
            nc.vector.tensor_copy(out=rbb, in_=ps_rb[:B, :RB])
            # splay [B, RB] (partition-major) into a single-partition row so it
            # can be the K=1 lhsT of the bias-augment matmul below
            for b in range(B):
                nc.sync.dma_start(
                    out=rbrow[:, b * RB : (b + 1) * RB], in_=rbb[b : b + 1, :]
                )

            # --- U1T shard load: [j, r, i], per j-chunk ---
            u1_tiles = []
            for jc in range(JC):
                u1t_t = dpool.tile([P, RB, D], fp32, tag=f"u1_{jc}")
                nc.sync.dma_start(out=u1t_t, in_=u1t_d.ap()[jc * P : (jc + 1) * P])
                u1_tiles.append(u1t_t)

            # --- V[i, b] per (r, ic): contract j on TensorE ---
            for r in range(RB):
                for ic in range(IC):
                    ps_v = ppool.tile([P, 512], fp32, tag="ps")
                    for jc in range(JC):
                        nc.tensor.matmul(
                            ps_v[:P, :B],
                            u1_tiles[jc][:, r, ic * P : (ic + 1) * P],
                            hbbarT[:, jc * B : (jc + 1) * B],
                            start=(jc == 0),
                            stop=(jc == JC - 1),
                        )
                    nc.vector.tensor_copy(out=vass[:, ic, :, r], in_=ps_v[:P, :B])

            # --- fold rf: vass[:, ic, b, :] += U2a[i, :] ---
            for ic in range(IC):
                for b in range(B):
                    nc.vector.tensor_add(
                        out=vass[:, ic, b, :],
                        in0=vass[:, ic, b, :],
                        in1=u2sb[:, ic, 0:RB],
                    )

            # --- hfT load ---
            hft_tiles = []
            for b in range(B):
                hft_t = dpool.tile([P, IC, S], fp32, tag=f"hft{b}", bufs=2)
                nc.sync.dma_start(
                    out=hft_t, in_=hft_d.ap()[b].rearrange("(ic p) x -> p ic x", p=P)
                )
                hft_tiles.append(hft_t)

            # --- out[r, x] per b: contract i; K=1 augment adds rbbar+bias ---
            for b in range(B):
                ps_o = ppool.tile([P, 512], fp32, tag="ps")
                for ic in range(IC):
                    nc.tensor.matmul(
                        ps_o[:RB, :S],
                        vass[:, ic, b, :],
                        hft_tiles[b][:, ic, :],
                        start=(ic == 0),
                        stop=False,
                    )
                nc.tensor.matmul(
                    ps_o[:RB, :S],
                    rbrow[:, b * RB : (b + 1) * RB],
                    ones1,
                    start=False,
                    stop=True,
                )
                out_sb_b = dpool.tile([RB, S], fp32, tag=f"out{b}", bufs=2)
                nc.vector.tensor_copy(out=out_sb_b, in_=ps_o[:RB, :S])
                nc.sync.dma_start(out=out_d.ap()[b], in_=out_sb_b)


def _get_nc(n_repeat=1):
    if n_repeat not in _NC_CACHE:
        _NC_CACHE[n_repeat] = _build_nc(n_repeat)
    return _NC_CACHE[n_repeat]


def _prep_inputs(h_forward, h_backward, U_1, U_2, bias):
    hf = np.ascontiguousarray(np.asarray(h_forward, dtype=np.float32))
    hb = np.ascontiguousarray(np.asarray(h_backward, dtype=np.float32))
    u1 = np.asarray(U_1, dtype=np.float32)
    u2 = np.asarray(U_2, dtype=np.float32)
    bz = np.asarray(bias, dtype=np.float32)

    hft = np.ascontiguousarray(hf.transpose(0, 2, 1))  # [B, i, x]

    in_maps = []
    for c in range(NCORES):
        rs = slice(c * RB, (c + 1) * RB)
        u1t_c = np.ascontiguousarray(u1[:, rs, :].transpose(2, 1, 0))  # [j, r, i]
        u2t_c = np.ascontiguousarray(
            np.concatenate([u2[:D, rs].T, u2[D:, rs].T], axis=0)
        )  # [2*RB, D]: rows 0:RB = U2a.T, RB:2RB = U2b.T
        bias_c = np.ascontiguousarray(bz[rs].reshape(1, RB))
        hb_c = np.ascontiguousarray(hb[:, c * SY : (c + 1) * SY, :])
        in_maps.append(
            {
                "hft": hft,
                "hb": hb_c,
                "u1t": u1t_c,
                "u2t": u2t_c,
                "biasr": bias_c,
            }
        )
    return in_maps


def kernel(h_forward, h_backward, U_1, U_2, bias):
    global LAST_RESULTS
    from concourse.bass_utils import run_bass_kernel_spmd

    nc = _get_nc()
    in_maps = _prep_inputs(h_forward, h_backward, U_1, U_2, bias)
    res = run_bass_kernel_spmd(
        nc, in_maps, core_ids=list(range(NCORES)), trace=TRACE
    )
    LAST_RESULTS = res
    shards = [res.results[c]["out"] for c in range(NCORES)]  # each [B, RB, S]
    out = np.concatenate(shards, axis=1)  # [B, R, S]
    return np.ascontiguousarray(out.transpose(0, 2, 1))  # [B, S, R]
